# revision 4
# baseline (speedup 1.0000x reference)
"""MoE BaseRouter (router MLP + top-2 dispatch/combine) on 8 TRN2 NeuronCores.

Strategy (data-parallel over tokens, per sharding hint):
  - 4096 tokens sharded 512/core. Each core runs the router MLP
    (x @ w1.T -> relu -> @ w2.T) in fp32 (float32r PE mode, full rate),
    softmax + top-2 via the DVE max8/max_index instructions.
  - Capacity-slot positions: per-core inclusive cumsum over local tokens via
    an upper-triangular matmul; cross-core exclusive per-expert offsets via a
    tiny (320 B) AllReduce of masked per-core expert totals. Expert usage /
    router-prob sums for the aux loss ride in the same AllReduce.
  - dispatch/combine are huge (2 x 201 MB) but ~0.02% nonzero. ExternalOutput
    buffers are pre-zeroed by run_bass_kernel_spmd (documented contract), so
    each core scatter-writes only its 1024 (dispatch,combine) pairs via
    indirect DMA; capacity-overflow entries are suppressed with an
    out-of-bounds sentinel index + bounds_check.

kernel(**inputs) takes the FULL inputs and returns the full
(dispatch, combine, router_probs, aux_loss) tuple, matching reference().
"""

import sys

if "/opt/trn_rl_repo" not in sys.path:
    sys.path.insert(0, "/opt/trn_rl_repo")

import numpy as np

H = 2048          # hidden
E = 8             # experts
K = 2             # top-k
CAP = 1536        # capacity = int(4096 * 1.5 * 2 / 8)
NCORES = 8
TL = 512          # tokens per core
NCH = 4           # token chunks of 128 per core
NFLAT = TL * E * CAP  # flat (token, expert, cap) slots per core
OOB = 2e7         # sentinel added to invalid (pos >= CAP) indices
AUX_SCALE = float(E) / (4096.0 * 4096.0 * K)  # == 2**-22
MM1_MODE = "fp32"  # "fp32" (exact, 4 cyc/row) | "bf16x3" (hi*hi+hi*lo+lo*hi, 3 cyc/row)

_CACHE = {}


def _build_nc():
    import concourse.mybir as mybir
    from concourse import bacc
    from concourse.bass import IndirectOffsetOnAxis
    from concourse.masks import make_upper_triangular
    from concourse.tile import TileContext

    f32 = mybir.dt.float32
    i32 = mybir.dt.int32
    u32 = mybir.dt.uint32
    Alu = mybir.AluOpType
    Act = mybir.ActivationFunctionType
    AX = mybir.AxisListType

    nc = bacc.Bacc(None, target_bir_lowering=False, debug=False)

    bf16 = mybir.dt.bfloat16
    if MM1_MODE == "fp32":
        x_ins = [nc.declare_dram_parameter("x", [128, 16, TL], f32, isOutput=False)]
        w1_ins = [nc.declare_dram_parameter("w1", [16, 128, 16, 128], f32, isOutput=False)]
    else:
        x_ins = [
            nc.declare_dram_parameter("x_hi", [128, 16, TL], bf16, isOutput=False),
            nc.declare_dram_parameter("x_lo", [128, 16, TL], bf16, isOutput=False),
        ]
        w1_ins = [
            nc.declare_dram_parameter("w1_hi", [16, 128, 16, 128], bf16, isOutput=False),
            nc.declare_dram_parameter("w1_lo", [16, 128, 16, 128], bf16, isOutput=False),
        ]
    w2_in = nc.declare_dram_parameter("w2", [128, 16, E], f32, isOutput=False)
    b1_in = nc.declare_dram_parameter("b1", [128, 16], f32, isOutput=False)
    b2_in = nc.declare_dram_parameter("b2", [128, E], f32, isOutput=False)
    maska_in = nc.declare_dram_parameter("maska", [E, 1], f32, isOutput=False)
    sel_in = nc.declare_dram_parameter("sel", [128, 1], f32, isOutput=False)

    dc_out = nc.declare_dram_parameter("dc", [NFLAT, 2], f32, isOutput=True)
    probs_out = nc.declare_dram_parameter("probs", [TL, E], f32, isOutput=True)
    aux_out = nc.declare_dram_parameter("aux", [1, 1], f32, isOutput=True)

    with TileContext(nc) as tc:
        with (
            tc.tile_pool(name="const", bufs=1) as cpool,
            tc.tile_pool(name="big", bufs=1) as bpool,
            tc.tile_pool(name="w1s", bufs=3) as wpool,
            tc.tile_pool(name="small", bufs=2) as spool,
            tc.tile_pool(name="psum", bufs=1, space="PSUM") as ppool,
            tc.tile_pool(name="dram", bufs=1, space="DRAM") as dpool,
        ):
            # ---------------- constants ----------------
            iota_i = cpool.tile([128, E], i32)
            nc.gpsimd.iota(iota_i[:], pattern=[[1, E]], base=0, channel_multiplier=0)
            iota_f = cpool.tile([128, E], f32)
            nc.vector.tensor_copy(iota_f[:], iota_i[:])

            tokb_i = cpool.tile([128, NCH], i32)
            for j in range(NCH):
                # flat slot base of token (j*128 + p): (j*128 + p) * E*CAP
                nc.gpsimd.iota(
                    tokb_i[:, j : j + 1],
                    pattern=[[1, 1]],
                    base=j * 128 * E * CAP,
                    channel_multiplier=E * CAP,
                )
            tokb_f = cpool.tile([128, NCH], f32)
            nc.vector.tensor_copy(tokb_f[:], tokb_i[:])

            u_incl = cpool.tile([128, 128], f32)
            make_upper_triangular(nc, u_incl[:], val=1.0, diag=True)
            ones128 = cpool.tile([128, 128], f32)
            nc.vector.memset(ones128[:], 1.0)

            # ---------------- inputs to SBUF ----------------
            x_dt = f32 if MM1_MODE == "fp32" else bf16
            x_sbs = []
            for xi, x_in in enumerate(x_ins):
                x_sb = bpool.tile([128, 16, TL], x_dt, name=f"x_sb{xi}")
                for q in range(4):
                    nc.sync.dma_start(
                        out=x_sb[:, 4 * q : 4 * q + 4, :],
                        in_=x_in[:, 4 * q : 4 * q + 4, :],
                    )
                x_sbs.append(x_sb)
            w2_sb = cpool.tile([128, 16, E], f32)
            nc.sync.dma_start(out=w2_sb[:], in_=w2_in[:])
            b1_sb = cpool.tile([128, 16], f32)
            nc.sync.dma_start(out=b1_sb[:], in_=b1_in[:])
            b2_sb = cpool.tile([128, E], f32)
            nc.sync.dma_start(out=b2_sb[:], in_=b2_in[:])
            maska_sb = cpool.tile([E, 1], f32)
            nc.sync.dma_start(out=maska_sb[:], in_=maska_in[:])
            sel_sb = cpool.tile([128, 1], f32)
            nc.sync.dma_start(out=sel_sb[:], in_=sel_in[:])

            # ---------------- phase 1: router MLP ----------------
            h_sb = bpool.tile([128, 16, TL], f32)  # h.T tiles: [o(part), ot, t]
            psum2 = [
                ppool.tile([128, E], f32, tag="mm2", bufs=NCH, name=f"psum2_{j}")
                for j in range(NCH)
            ]
            for ot in range(16):
                w1ts = []
                for wi, w1_in in enumerate(w1_ins):
                    w1t = wpool.tile(
                        [128, 16, 128], x_dt, tag=f"w1t{wi}", name=f"w1t{wi}_{ot}"
                    )
                    nc.sync.dma_start(out=w1t[:], in_=w1_in[ot])
                    w1ts.append(w1t)
                ps = ppool.tile([128, TL], f32, tag="mm1", bufs=2, name=f"ps1_{ot}")
                if MM1_MODE == "fp32":
                    terms = [(w1ts[0], x_sbs[0])] * 16
                    for i in range(16):
                        nc.tensor.matmul(
                            ps[:],
                            lhsT=w1ts[0][:, i, :],
                            rhs=x_sbs[0][:, i, :],
                            start=(i == 0),
                            stop=(i == 15),
                        )
                else:
                    # h = x_hi@w_hi + x_hi@w_lo + x_lo@w_hi (lo*lo dropped)
                    n_mm = 0
                    for i in range(16):
                        for wt, xt in (
                            (w1ts[0], x_sbs[0]),
                            (w1ts[1], x_sbs[0]),
                            (w1ts[0], x_sbs[1]),
                        ):
                            nc.tensor.matmul(
                                ps[:],
                                lhsT=wt[:, i, :],
                                rhs=xt[:, i, :],
                                start=(n_mm == 0),
                                stop=(n_mm == 47),
                            )
                            n_mm += 1
                nc.scalar.activation(
                    out=h_sb[:, ot, :],
                    in_=ps[:],
                    func=Act.Relu,
                    bias=b1_sb[:, ot : ot + 1],
                    scale=1.0,
                )
                # interleaved second matmul: logits[t, e] += h[t, hdim]*w2[e, hdim]
                for j in range(NCH):
                    nc.tensor.matmul(
                        psum2[j][:],
                        lhsT=h_sb[:, ot, j * 128 : (j + 1) * 128],
                        rhs=w2_sb[:, ot, :],
                        start=(ot == 0),
                        stop=(ot == 15),
                        skip_group_check=True,
                    )

            # ---------------- phase 2: per-chunk routing ----------------
            probs_all = bpool.tile([128, NCH, E], f32)
            eq0_all = bpool.tile([128, NCH, E], f32)
            eq1_all = bpool.tile([128, NCH, E], f32)
            cbef_all = bpool.tile([128, NCH, E], f32)
            if_all = bpool.tile([128, NCH, K], f32)
            val2_all = bpool.tile([128, NCH, K, 2], f32)
            nc.vector.memset(val2_all[:, :, :, 0:1], 1.0)  # dispatch value
            acc_sb = bpool.tile([128, E], f32)  # running expert totals (bcast)
            spr_sb = bpool.tile([1, E], f32)  # running sum of probs

            for j in range(NCH):
                logits = spool.tile([128, E], f32, tag="logits", bufs=2)
                nc.vector.tensor_tensor(
                    out=logits[:], in0=psum2[j][:], in1=b2_sb[:], op=Alu.add
                )
                mx = spool.tile([128, 1], f32, tag="mx", bufs=2)
                nc.vector.reduce_max(out=mx[:], in_=logits[:], axis=AX.X)
                nmx = spool.tile([128, 1], f32, tag="nmx", bufs=2)
                nc.vector.tensor_scalar_mul(nmx[:], mx[:], -1.0)
                pexp = spool.tile([128, E], f32, tag="pexp", bufs=2)
                nc.scalar.activation(
                    out=pexp[:], in_=logits[:], func=Act.Exp, bias=nmx[:, 0:1], scale=1.0
                )
                sm = spool.tile([128, 1], f32, tag="sm", bufs=2)
                nc.vector.reduce_sum(out=sm[:], in_=pexp[:], axis=AX.X)
                rsm = spool.tile([128, 1], f32, tag="rsm", bufs=2)
                nc.vector.reciprocal(rsm[:], sm[:])
                nc.vector.tensor_scalar_mul(probs_all[:, j, :], pexp[:], rsm[:, 0:1])
                nc.sync.dma_start(
                    out=probs_out[j * 128 : (j + 1) * 128, :], in_=probs_all[:, j, :]
                )

                top8 = spool.tile([128, 8], f32, tag="top8", bufs=2)
                nc.vector.max(out=top8[:], in_=probs_all[:, j, :])
                idx8 = spool.tile([128, 8], u32, tag="idx8", bufs=2)
                nc.vector.max_index(out=idx8[:], in_max=top8[:], in_values=probs_all[:, j, :])

                # renormalized top-2 probs -> combine values
                den = spool.tile([128, 1], f32, tag="den", bufs=2)
                nc.vector.scalar_tensor_tensor(
                    out=den[:], in0=top8[:, 0:1], scalar=1e-8,
                    in1=top8[:, 1:2], op0=Alu.add, op1=Alu.add,
                )
                rden = spool.tile([128, 1], f32, tag="rden", bufs=2)
                nc.vector.reciprocal(rden[:], den[:])
                nc.vector.tensor_tensor(
                    out=val2_all[:, j, 0, 1:2], in0=top8[:, 0:1], in1=rden[:], op=Alu.mult
                )
                nc.vector.tensor_tensor(
                    out=val2_all[:, j, 1, 1:2], in0=top8[:, 1:2], in1=rden[:], op=Alu.mult
                )

                # expert ids as f32 + one-hots
                nc.vector.tensor_copy(if_all[:, j, 0:1], idx8[:, 0:1])
                nc.vector.tensor_copy(if_all[:, j, 1:2], idx8[:, 1:2])
                nc.vector.tensor_tensor(
                    out=eq0_all[:, j, :], in0=iota_f[:],
                    in1=if_all[:, j, 0:1].to_broadcast([128, E]), op=Alu.is_equal,
                )
                nc.vector.tensor_tensor(
                    out=eq1_all[:, j, :], in0=iota_f[:],
                    in1=if_all[:, j, 1:2].to_broadcast([128, E]), op=Alu.is_equal,
                )
                oh = spool.tile([128, E], f32, tag="oh", bufs=2)
                nc.vector.tensor_tensor(
                    out=oh[:], in0=eq0_all[:, j, :], in1=eq1_all[:, j, :], op=Alu.add
                )

                # local inclusive cumsum + chunk totals (broadcast to 128 parts)
                cntu = ppool.tile([128, E], f32, tag="auxp", bufs=2, name=f"cntu_{j}")
                nc.tensor.matmul(cntu[:], lhsT=u_incl[:], rhs=oh[:], start=True, stop=True)
                totb = ppool.tile([128, E], f32, tag="auxp", bufs=2, name=f"totb_{j}")
                nc.tensor.matmul(totb[:], lhsT=ones128[:], rhs=oh[:], start=True, stop=True)
                sprow = ppool.tile([1, E], f32, tag="auxp", bufs=2, name=f"sprow_{j}")
                nc.tensor.matmul(
                    sprow[:], lhsT=ones128[:, 0:1], rhs=probs_all[:, j, :],
                    start=True, stop=True,
                )

                # cnt_before = (inclusive - own) + totals of earlier chunks
                nc.vector.tensor_tensor(
                    out=cbef_all[:, j, :], in0=cntu[:], in1=oh[:], op=Alu.subtract
                )
                if j > 0:
                    nc.vector.tensor_tensor(
                        out=cbef_all[:, j, :], in0=cbef_all[:, j, :], in1=acc_sb[:],
                        op=Alu.add,
                    )
                    nc.vector.tensor_tensor(
                        out=acc_sb[:], in0=acc_sb[:], in1=totb[:], op=Alu.add
                    )
                    nc.vector.tensor_tensor(
                        out=spr_sb[:], in0=spr_sb[:], in1=sprow[:], op=Alu.add
                    )
                else:
                    nc.vector.tensor_copy(acc_sb[:], totb[:])
                    nc.vector.tensor_copy(spr_sb[:], sprow[:])

            # ---------------- phase 3: tiny AllReduce ----------------
            contrib = spool.tile([E, E], f32)  # [dst_core, expert]
            nc.vector.tensor_scalar_mul(contrib[:], acc_sb[0:E, :], maska_sb[:, 0:1])

            ar_in = dpool.tile([80], f32)
            ar_out = dpool.tile([80], f32, addr_space="Shared")
            nc.sync.dma_start(
                out=ar_in[0:64].rearrange("(c e) -> c e", e=E), in_=contrib[:]
            )
            nc.sync.dma_start(
                out=ar_in[64:72].rearrange("(a e) -> a e", a=1), in_=acc_sb[0:1, :]
            )
            nc.sync.dma_start(
                out=ar_in[72:80].rearrange("(a e) -> a e", a=1), in_=spr_sb[:]
            )
            nc.gpsimd.collective_compute(
                "AllReduce",
                Alu.add,
                ins=[ar_in[:].opt()],
                outs=[ar_out[:].opt()],
                replica_groups=[list(range(NCORES))],
            )

            # ---------------- phase 4: offsets + aux ----------------
            a128 = spool.tile([128, E], f32)
            nc.vector.memset(a128[:], 0.0)
            nc.sync.dma_start(
                out=a128[0:E, :], in_=ar_out[0:64].rearrange("(c e) -> c e", e=E)
            )
            g_sb = spool.tile([1, 2 * E], f32)
            nc.sync.dma_start(
                out=g_sb[:], in_=ar_out[64:80].rearrange("(a x) -> a x", a=1)
            )

            selmat = spool.tile([128, 128], f32)
            nc.vector.tensor_scalar_mul(selmat[:], ones128[:], sel_sb[:, 0:1])
            offs = ppool.tile([128, E], f32, tag="auxp", bufs=2)
            nc.tensor.matmul(offs[:], lhsT=selmat[:], rhs=a128[:], start=True, stop=True)

            auxv = spool.tile([1, E], f32)
            nc.vector.tensor_tensor(
                out=auxv[:], in0=g_sb[:, 0:E], in1=g_sb[:, E : 2 * E], op=Alu.mult
            )
            auxs = spool.tile([1, 1], f32)
            nc.vector.reduce_sum(out=auxs[:], in_=auxv[:], axis=AX.X)
            nc.vector.tensor_scalar_mul(auxs[:], auxs[:], AUX_SCALE)
            nc.sync.dma_start(out=aux_out[:], in_=auxs[:])

            # ---------------- phase 5: positions + scatter ----------------
            idx_i = bpool.tile([128, NCH, K], i32)
            for j in range(NCH):
                cplus = spool.tile([128, E], f32, tag="cplus", bufs=2)
                nc.vector.tensor_tensor(
                    out=cplus[:], in0=cbef_all[:, j, :], in1=offs[:], op=Alu.add
                )
                for k, eqk in ((0, eq0_all), (1, eq1_all)):
                    tmp8 = spool.tile([128, E], f32, tag="tmp8", bufs=2)
                    posk = spool.tile([128, 1], f32, tag="posk", bufs=2)
                    nc.vector.scalar_tensor_tensor(
                        out=tmp8[:], in0=cplus[:], scalar=0.0, in1=eqk[:, j, :],
                        op0=Alu.add, op1=Alu.mult, accum_out=posk[:],
                    )
                    idxf = spool.tile([128, 1], f32, tag="idxf", bufs=2)
                    nc.vector.scalar_tensor_tensor(
                        out=idxf[:], in0=if_all[:, j, k : k + 1], scalar=float(CAP),
                        in1=tokb_f[:, j : j + 1], op0=Alu.mult, op1=Alu.add,
                    )
                    nc.vector.tensor_tensor(
                        out=idxf[:], in0=idxf[:], in1=posk[:], op=Alu.add
                    )
                    over = spool.tile([128, 1], f32, tag="over", bufs=2)
                    nc.vector.tensor_scalar(
                        out=over[:], in0=posk[:], scalar1=CAP - 0.5, scalar2=None,
                        op0=Alu.is_gt,
                    )
                    nc.vector.scalar_tensor_tensor(
                        out=idxf[:], in0=over[:], scalar=OOB, in1=idxf[:],
                        op0=Alu.mult, op1=Alu.add,
                    )
                    nc.vector.tensor_copy(idx_i[:, j, k : k + 1], idxf[:])
                    nc.gpsimd.indirect_dma_start(
                        out=dc_out[:, :],
                        out_offset=IndirectOffsetOnAxis(ap=idx_i[:, j, k : k + 1], axis=0),
                        in_=val2_all[:, j, k, :],
                        in_offset=None,
                        bounds_check=NFLAT - 1,
                        oob_is_err=False,
                    )

    nc.finalize()  # run bacc passes (register allocation etc.)
    return nc


def _get_nc():
    if "nc" not in _CACHE:
        _CACHE["nc"] = _build_nc()
    return _CACHE["nc"]


def _prep_in_maps(hidden_states, w1, b1, w2, b2):
    x = np.ascontiguousarray(np.asarray(hidden_states, np.float32)).reshape(-1, H)
    w1 = np.asarray(w1, np.float32)
    b1 = np.asarray(b1, np.float32)
    w2 = np.asarray(w2, np.float32)
    b2 = np.asarray(b2, np.float32)

    # weights in PE-friendly layouts (pure host-side input prep)
    import ml_dtypes

    w1_prep = np.ascontiguousarray(
        w1.reshape(16, 128, 16, 128).transpose(0, 3, 2, 1)
    )  # [ot, p(=h%128), i(=h//128), oi]
    if MM1_MODE == "bf16x3":
        w1_hi = w1_prep.astype(ml_dtypes.bfloat16)
        w1_lo = (w1_prep - w1_hi.astype(np.float32)).astype(ml_dtypes.bfloat16)
    w2_prep = np.ascontiguousarray(w2.reshape(E, 16, 128).transpose(2, 1, 0))
    b1_prep = np.ascontiguousarray(b1.reshape(16, 128).T)
    b2_prep = np.ascontiguousarray(np.tile(b2[None, :], (128, 1)))

    in_maps = []
    for c in range(NCORES):
        xs = x[c * TL : (c + 1) * TL]  # [512, 2048]
        x_prep = np.ascontiguousarray(xs.reshape(TL, 16, 128).transpose(2, 1, 0))
        maska = (np.arange(E) > c).astype(np.float32).reshape(E, 1)
        sel = np.zeros((128, 1), np.float32)
        sel[c, 0] = 1.0
        if MM1_MODE == "bf16x3":
            x_hi = x_prep.astype(ml_dtypes.bfloat16)
            x_lo = (x_prep - x_hi.astype(np.float32)).astype(ml_dtypes.bfloat16)
            mm1_inputs = {"x_hi": x_hi, "x_lo": x_lo, "w1_hi": w1_hi, "w1_lo": w1_lo}
        else:
            mm1_inputs = {"x": x_prep, "w1": w1_prep}
        in_maps.append(
            {
                **mm1_inputs,
                "w2": w2_prep,
                "b1": b1_prep,
                "b2": b2_prep,
                "maska": np.ascontiguousarray(maska),
                "sel": sel,
            }
        )
    return in_maps


def _assemble(results):
    disp = np.empty((NCORES * TL, E, CAP), np.float32)
    comb = np.empty((NCORES * TL, E, CAP), np.float32)
    probs = np.empty((NCORES * TL, E), np.float32)
    for c, r in enumerate(results):
        dc = np.asarray(r["dc"]).reshape(TL, E, CAP, 2)
        disp[c * TL : (c + 1) * TL] = dc[..., 0]
        comb[c * TL : (c + 1) * TL] = dc[..., 1]
        probs[c * TL : (c + 1) * TL] = np.asarray(r["probs"])
    aux = np.float32(np.asarray(results[0]["aux"]).reshape(-1)[0])
    return (
        disp.reshape(2, 2048, E, CAP),
        comb.reshape(2, 2048, E, CAP),
        probs.reshape(2, 2048, E),
        aux,
    )


def run(trace=False, **inputs):
    """Run on the 8 NeuronCores; returns (outputs_tuple, exec_time_ns|None)."""
    from concourse.bass_utils import run_bass_kernel_spmd

    nc = _get_nc()
    in_maps = _prep_in_maps(
        inputs["hidden_states"], inputs["w1"], inputs["b1"], inputs["w2"], inputs["b2"]
    )
    res = run_bass_kernel_spmd(nc, in_maps, core_ids=list(range(NCORES)), trace=trace)
    return _assemble(res.results), res.exec_time_ns


def kernel(**inputs):
    outputs, _ = run(trace=False, **inputs)
    return outputs


# revision 5
# speedup vs baseline: 1.2401x; 1.2401x over previous
"""MoE BaseRouter (router MLP + top-2 dispatch/combine) on 8 TRN2 NeuronCores.

Strategy (data-parallel over tokens, per sharding hint):
  - 4096 tokens sharded 512/core. Each core runs the router MLP
    (x @ w1.T -> relu -> @ w2.T) in fp32 (float32r PE mode, full rate),
    softmax + top-2 via the DVE max8/max_index instructions.
  - Capacity-slot positions: per-core inclusive cumsum over local tokens via
    an upper-triangular matmul; cross-core exclusive per-expert offsets via a
    tiny (320 B) AllReduce of masked per-core expert totals. Expert usage /
    router-prob sums for the aux loss ride in the same AllReduce.
  - dispatch/combine are huge (2 x 201 MB) but ~0.02% nonzero. ExternalOutput
    buffers are pre-zeroed by run_bass_kernel_spmd (documented contract), so
    each core scatter-writes only its 1024 (dispatch,combine) pairs via
    indirect DMA; capacity-overflow entries are suppressed with an
    out-of-bounds sentinel index + bounds_check.

kernel(**inputs) takes the FULL inputs and returns the full
(dispatch, combine, router_probs, aux_loss) tuple, matching reference().
"""

import sys

if "/opt/trn_rl_repo" not in sys.path:
    sys.path.insert(0, "/opt/trn_rl_repo")

import numpy as np

H = 2048          # hidden
E = 8             # experts
K = 2             # top-k
CAP = 1536        # capacity = int(4096 * 1.5 * 2 / 8)
NCORES = 8
TL = 512          # tokens per core
NCH = 4           # token chunks of 128 per core
NFLAT = TL * E * CAP  # flat (token, expert, cap) slots per core
OOB = 2e7         # sentinel added to invalid (pos >= CAP) indices
AUX_SCALE = float(E) / (4096.0 * 4096.0 * K)  # == 2**-22
MM1_MODE = "bf16x3"  # "fp32" (exact, 4 cyc/row) | "bf16x3" (hi*hi+hi*lo+lo*hi, 3 cyc/row)

_CACHE = {}


def _build_nc():
    import concourse.mybir as mybir
    from concourse import bacc
    from concourse.bass import IndirectOffsetOnAxis
    from concourse.masks import make_upper_triangular
    from concourse.tile import TileContext

    f32 = mybir.dt.float32
    i32 = mybir.dt.int32
    u32 = mybir.dt.uint32
    Alu = mybir.AluOpType
    Act = mybir.ActivationFunctionType
    AX = mybir.AxisListType

    nc = bacc.Bacc(None, target_bir_lowering=False, debug=False)

    bf16 = mybir.dt.bfloat16
    if MM1_MODE == "fp32":
        x_ins = [nc.declare_dram_parameter("x", [128, 16, TL], f32, isOutput=False)]
        w1_ins = [nc.declare_dram_parameter("w1", [16, 128, 16, 128], f32, isOutput=False)]
    else:
        x_ins = [
            nc.declare_dram_parameter("x_hi", [128, 16, TL], bf16, isOutput=False),
            nc.declare_dram_parameter("x_lo", [128, 16, TL], bf16, isOutput=False),
        ]
        w1_ins = [
            nc.declare_dram_parameter("w1_hi", [16, 128, 16, 128], bf16, isOutput=False),
            nc.declare_dram_parameter("w1_lo", [16, 128, 16, 128], bf16, isOutput=False),
        ]
    w2_in = nc.declare_dram_parameter("w2", [128, 16, E], f32, isOutput=False)
    b1_in = nc.declare_dram_parameter("b1", [128, 16], f32, isOutput=False)
    b2_in = nc.declare_dram_parameter("b2", [128, E], f32, isOutput=False)
    maska_in = nc.declare_dram_parameter("maska", [E, 1], f32, isOutput=False)
    sel_in = nc.declare_dram_parameter("sel", [128, 1], f32, isOutput=False)

    dc_out = nc.declare_dram_parameter("dc", [NFLAT, 2], f32, isOutput=True)
    probs_out = nc.declare_dram_parameter("probs", [TL, E], f32, isOutput=True)
    aux_out = nc.declare_dram_parameter("aux", [1, 1], f32, isOutput=True)

    with TileContext(nc) as tc:
        with (
            tc.tile_pool(name="const", bufs=1) as cpool,
            tc.tile_pool(name="big", bufs=1) as bpool,
            tc.tile_pool(name="w1s", bufs=3) as wpool,
            tc.tile_pool(name="small", bufs=2) as spool,
            tc.tile_pool(name="psum", bufs=1, space="PSUM") as ppool,
            tc.tile_pool(name="dram", bufs=1, space="DRAM") as dpool,
        ):
            # ---------------- constants ----------------
            iota_i = cpool.tile([128, E], i32)
            nc.gpsimd.iota(iota_i[:], pattern=[[1, E]], base=0, channel_multiplier=0)
            iota_f = cpool.tile([128, E], f32)
            nc.vector.tensor_copy(iota_f[:], iota_i[:])

            tokb_i = cpool.tile([128, NCH], i32)
            for j in range(NCH):
                # flat slot base of token (j*128 + p): (j*128 + p) * E*CAP
                nc.gpsimd.iota(
                    tokb_i[:, j : j + 1],
                    pattern=[[1, 1]],
                    base=j * 128 * E * CAP,
                    channel_multiplier=E * CAP,
                )
            tokb_f = cpool.tile([128, NCH], f32)
            nc.vector.tensor_copy(tokb_f[:], tokb_i[:])

            u_incl = cpool.tile([128, 128], f32)
            make_upper_triangular(nc, u_incl[:], val=1.0, diag=True)
            ones128 = cpool.tile([128, 128], f32)
            nc.vector.memset(ones128[:], 1.0)

            # ---------------- inputs to SBUF ----------------
            # first w1 slab loads first so the PE can start ASAP; x chunk 0
            # next; the rest stream behind.
            x_dt = f32 if MM1_MODE == "fp32" else bf16
            w1t0s = []
            for wi, w1_in in enumerate(w1_ins):
                w1t = wpool.tile([128, 16, 128], x_dt, tag=f"w1t{wi}", name=f"w1t{wi}_0")
                nc.sync.dma_start(out=w1t[:], in_=w1_in[0])
                w1t0s.append(w1t)
            x_sbs = []
            for xi, x_in in enumerate(x_ins):
                x_sb = bpool.tile([128, 16, TL], x_dt, name=f"x_sb{xi}")
                x_sbs.append(x_sb)
            for q in range(8):
                for xi, x_in in enumerate(x_ins):
                    nc.sync.dma_start(
                        out=x_sbs[xi][:, 2 * q : 2 * q + 2, :],
                        in_=x_in[:, 2 * q : 2 * q + 2, :],
                    )
            w2_sb = cpool.tile([128, 16, E], f32)
            nc.sync.dma_start(out=w2_sb[:], in_=w2_in[:])
            b1_sb = cpool.tile([128, 16], f32)
            nc.sync.dma_start(out=b1_sb[:], in_=b1_in[:])
            b2_sb = cpool.tile([128, E], f32)
            nc.sync.dma_start(out=b2_sb[:], in_=b2_in[:])
            maska_sb = cpool.tile([E, 1], f32)
            nc.sync.dma_start(out=maska_sb[:], in_=maska_in[:])
            sel_sb = cpool.tile([128, 1], f32)
            nc.sync.dma_start(out=sel_sb[:], in_=sel_in[:])

            # ---------------- phase 1: router MLP ----------------
            h_sb = bpool.tile([128, 16, TL], f32)  # h.T tiles: [o(part), ot, t]
            psum2 = [
                ppool.tile([128, E], f32, tag="mm2", bufs=NCH, name=f"psum2_{j}")
                for j in range(NCH)
            ]
            for ot in range(16):
                if ot == 0:
                    w1ts = w1t0s
                else:
                    w1ts = []
                    for wi, w1_in in enumerate(w1_ins):
                        w1t = wpool.tile(
                            [128, 16, 128], x_dt, tag=f"w1t{wi}", name=f"w1t{wi}_{ot}"
                        )
                        nc.sync.dma_start(out=w1t[:], in_=w1_in[ot])
                        w1ts.append(w1t)
                ps = ppool.tile([128, TL], f32, tag="mm1", bufs=2, name=f"ps1_{ot}")
                if MM1_MODE == "fp32":
                    terms = [(w1ts[0], x_sbs[0])] * 16
                    for i in range(16):
                        nc.tensor.matmul(
                            ps[:],
                            lhsT=w1ts[0][:, i, :],
                            rhs=x_sbs[0][:, i, :],
                            start=(i == 0),
                            stop=(i == 15),
                        )
                else:
                    # h = x_hi@w_hi + x_hi@w_lo + x_lo@w_hi (lo*lo dropped)
                    n_mm = 0
                    for i in range(16):
                        for wt, xt in (
                            (w1ts[0], x_sbs[0]),
                            (w1ts[1], x_sbs[0]),
                            (w1ts[0], x_sbs[1]),
                        ):
                            nc.tensor.matmul(
                                ps[:],
                                lhsT=wt[:, i, :],
                                rhs=xt[:, i, :],
                                start=(n_mm == 0),
                                stop=(n_mm == 47),
                            )
                            n_mm += 1
                nc.scalar.activation(
                    out=h_sb[:, ot, :],
                    in_=ps[:],
                    func=Act.Relu,
                    bias=b1_sb[:, ot : ot + 1],
                    scale=1.0,
                )
                # interleaved second matmul: logits[t, e] += h[t, hdim]*w2[e, hdim]
                for j in range(NCH):
                    nc.tensor.matmul(
                        psum2[j][:],
                        lhsT=h_sb[:, ot, j * 128 : (j + 1) * 128],
                        rhs=w2_sb[:, ot, :],
                        start=(ot == 0),
                        stop=(ot == 15),
                        skip_group_check=True,
                    )

            # ---------------- phase 2: per-chunk routing ----------------
            probs_all = bpool.tile([128, NCH, E], f32)
            eq0_all = bpool.tile([128, NCH, E], f32)
            eq1_all = bpool.tile([128, NCH, E], f32)
            cbef_all = bpool.tile([128, NCH, E], f32)
            if_all = bpool.tile([128, NCH, K], f32)
            val2_all = bpool.tile([128, NCH, K, 2], f32)
            nc.vector.memset(val2_all[:, :, :, 0:1], 1.0)  # dispatch value
            acc_sb = bpool.tile([128, E], f32)  # running expert totals (bcast)
            spr_sb = bpool.tile([1, E], f32)  # running sum of probs

            for j in range(NCH):
                logits = spool.tile([128, E], f32, tag="logits", bufs=2)
                nc.vector.tensor_tensor(
                    out=logits[:], in0=psum2[j][:], in1=b2_sb[:], op=Alu.add
                )
                mx = spool.tile([128, 1], f32, tag="mx", bufs=2)
                nc.vector.reduce_max(out=mx[:], in_=logits[:], axis=AX.X)
                nmx = spool.tile([128, 1], f32, tag="nmx", bufs=2)
                nc.vector.tensor_scalar_mul(nmx[:], mx[:], -1.0)
                pexp = spool.tile([128, E], f32, tag="pexp", bufs=2)
                nc.scalar.activation(
                    out=pexp[:], in_=logits[:], func=Act.Exp, bias=nmx[:, 0:1], scale=1.0
                )
                sm = spool.tile([128, 1], f32, tag="sm", bufs=2)
                nc.vector.reduce_sum(out=sm[:], in_=pexp[:], axis=AX.X)
                rsm = spool.tile([128, 1], f32, tag="rsm", bufs=2)
                nc.vector.reciprocal(rsm[:], sm[:])
                nc.vector.tensor_scalar_mul(probs_all[:, j, :], pexp[:], rsm[:, 0:1])
                nc.sync.dma_start(
                    out=probs_out[j * 128 : (j + 1) * 128, :], in_=probs_all[:, j, :]
                )

                top8 = spool.tile([128, 8], f32, tag="top8", bufs=2)
                nc.vector.max(out=top8[:], in_=probs_all[:, j, :])
                idx8 = spool.tile([128, 8], u32, tag="idx8", bufs=2)
                nc.vector.max_index(out=idx8[:], in_max=top8[:], in_values=probs_all[:, j, :])

                # renormalized top-2 probs -> combine values
                den = spool.tile([128, 1], f32, tag="den", bufs=2)
                nc.vector.scalar_tensor_tensor(
                    out=den[:], in0=top8[:, 0:1], scalar=1e-8,
                    in1=top8[:, 1:2], op0=Alu.add, op1=Alu.add,
                )
                rden = spool.tile([128, 1], f32, tag="rden", bufs=2)
                nc.vector.reciprocal(rden[:], den[:])
                nc.vector.tensor_tensor(
                    out=val2_all[:, j, 0, 1:2], in0=top8[:, 0:1], in1=rden[:], op=Alu.mult
                )
                nc.vector.tensor_tensor(
                    out=val2_all[:, j, 1, 1:2], in0=top8[:, 1:2], in1=rden[:], op=Alu.mult
                )

                # expert ids as f32 + one-hots
                nc.vector.tensor_copy(if_all[:, j, 0:1], idx8[:, 0:1])
                nc.vector.tensor_copy(if_all[:, j, 1:2], idx8[:, 1:2])
                nc.vector.tensor_tensor(
                    out=eq0_all[:, j, :], in0=iota_f[:],
                    in1=if_all[:, j, 0:1].to_broadcast([128, E]), op=Alu.is_equal,
                )
                nc.vector.tensor_tensor(
                    out=eq1_all[:, j, :], in0=iota_f[:],
                    in1=if_all[:, j, 1:2].to_broadcast([128, E]), op=Alu.is_equal,
                )
                oh = spool.tile([128, E], f32, tag="oh", bufs=2)
                nc.vector.tensor_tensor(
                    out=oh[:], in0=eq0_all[:, j, :], in1=eq1_all[:, j, :], op=Alu.add
                )

                # local inclusive cumsum + chunk totals (broadcast to 128 parts)
                cntu = ppool.tile([128, E], f32, tag="auxp", bufs=2, name=f"cntu_{j}")
                nc.tensor.matmul(cntu[:], lhsT=u_incl[:], rhs=oh[:], start=True, stop=True)
                totb = ppool.tile([128, E], f32, tag="auxp", bufs=2, name=f"totb_{j}")
                nc.tensor.matmul(totb[:], lhsT=ones128[:], rhs=oh[:], start=True, stop=True)
                sprow = ppool.tile([1, E], f32, tag="auxp", bufs=2, name=f"sprow_{j}")
                nc.tensor.matmul(
                    sprow[:], lhsT=ones128[:, 0:1], rhs=probs_all[:, j, :],
                    start=True, stop=True,
                )

                # cnt_before = (inclusive - own) + totals of earlier chunks
                nc.vector.tensor_tensor(
                    out=cbef_all[:, j, :], in0=cntu[:], in1=oh[:], op=Alu.subtract
                )
                if j > 0:
                    nc.vector.tensor_tensor(
                        out=cbef_all[:, j, :], in0=cbef_all[:, j, :], in1=acc_sb[:],
                        op=Alu.add,
                    )
                    nc.vector.tensor_tensor(
                        out=acc_sb[:], in0=acc_sb[:], in1=totb[:], op=Alu.add
                    )
                    nc.vector.tensor_tensor(
                        out=spr_sb[:], in0=spr_sb[:], in1=sprow[:], op=Alu.add
                    )
                else:
                    nc.vector.tensor_copy(acc_sb[:], totb[:])
                    nc.vector.tensor_copy(spr_sb[:], sprow[:])

            # ---------------- phase 3: tiny AllReduce ----------------
            contrib = spool.tile([E, E], f32)  # [dst_core, expert]
            nc.vector.tensor_scalar_mul(contrib[:], acc_sb[0:E, :], maska_sb[:, 0:1])

            ar_in = dpool.tile([80], f32)
            ar_out = dpool.tile([80], f32, addr_space="Shared")
            nc.sync.dma_start(
                out=ar_in[0:64].rearrange("(c e) -> c e", e=E), in_=contrib[:]
            )
            nc.sync.dma_start(
                out=ar_in[64:72].rearrange("(a e) -> a e", a=1), in_=acc_sb[0:1, :]
            )
            nc.sync.dma_start(
                out=ar_in[72:80].rearrange("(a e) -> a e", a=1), in_=spr_sb[:]
            )
            nc.gpsimd.collective_compute(
                "AllReduce",
                Alu.add,
                ins=[ar_in[:].opt()],
                outs=[ar_out[:].opt()],
                replica_groups=[list(range(NCORES))],
            )

            # ---------------- phase 4: offsets + aux ----------------
            a128 = spool.tile([128, E], f32)
            nc.vector.memset(a128[:], 0.0)
            nc.sync.dma_start(
                out=a128[0:E, :], in_=ar_out[0:64].rearrange("(c e) -> c e", e=E)
            )
            g_sb = spool.tile([1, 2 * E], f32)
            nc.sync.dma_start(
                out=g_sb[:], in_=ar_out[64:80].rearrange("(a x) -> a x", a=1)
            )

            selmat = spool.tile([128, 128], f32)
            nc.vector.tensor_scalar_mul(selmat[:], ones128[:], sel_sb[:, 0:1])
            offs = ppool.tile([128, E], f32, tag="auxp", bufs=2)
            nc.tensor.matmul(offs[:], lhsT=selmat[:], rhs=a128[:], start=True, stop=True)

            auxv = spool.tile([1, E], f32)
            nc.vector.tensor_tensor(
                out=auxv[:], in0=g_sb[:, 0:E], in1=g_sb[:, E : 2 * E], op=Alu.mult
            )
            auxs = spool.tile([1, 1], f32)
            nc.vector.reduce_sum(out=auxs[:], in_=auxv[:], axis=AX.X)
            nc.vector.tensor_scalar_mul(auxs[:], auxs[:], AUX_SCALE)
            nc.sync.dma_start(out=aux_out[:], in_=auxs[:])

            # ---------------- phase 5: positions + scatter ----------------
            idx_i = bpool.tile([128, NCH, K], i32)
            for j in range(NCH):
                cplus = spool.tile([128, E], f32, tag="cplus", bufs=2)
                nc.vector.tensor_tensor(
                    out=cplus[:], in0=cbef_all[:, j, :], in1=offs[:], op=Alu.add
                )
                for k, eqk in ((0, eq0_all), (1, eq1_all)):
                    tmp8 = spool.tile([128, E], f32, tag="tmp8", bufs=2)
                    posk = spool.tile([128, 1], f32, tag="posk", bufs=2)
                    nc.vector.scalar_tensor_tensor(
                        out=tmp8[:], in0=cplus[:], scalar=0.0, in1=eqk[:, j, :],
                        op0=Alu.add, op1=Alu.mult, accum_out=posk[:],
                    )
                    idxf = spool.tile([128, 1], f32, tag="idxf", bufs=2)
                    nc.vector.scalar_tensor_tensor(
                        out=idxf[:], in0=if_all[:, j, k : k + 1], scalar=float(CAP),
                        in1=tokb_f[:, j : j + 1], op0=Alu.mult, op1=Alu.add,
                    )
                    nc.vector.tensor_tensor(
                        out=idxf[:], in0=idxf[:], in1=posk[:], op=Alu.add
                    )
                    over = spool.tile([128, 1], f32, tag="over", bufs=2)
                    nc.vector.tensor_scalar(
                        out=over[:], in0=posk[:], scalar1=CAP - 0.5, scalar2=None,
                        op0=Alu.is_gt,
                    )
                    nc.vector.scalar_tensor_tensor(
                        out=idxf[:], in0=over[:], scalar=OOB, in1=idxf[:],
                        op0=Alu.mult, op1=Alu.add,
                    )
                    nc.vector.tensor_copy(idx_i[:, j, k : k + 1], idxf[:])
                    nc.gpsimd.indirect_dma_start(
                        out=dc_out[:, :],
                        out_offset=IndirectOffsetOnAxis(ap=idx_i[:, j, k : k + 1], axis=0),
                        in_=val2_all[:, j, k, :],
                        in_offset=None,
                        bounds_check=NFLAT - 1,
                        oob_is_err=False,
                    )

    nc.finalize()  # run bacc passes (register allocation etc.)
    return nc


def _get_nc():
    if "nc" not in _CACHE:
        _CACHE["nc"] = _build_nc()
    return _CACHE["nc"]


def _prep_in_maps(hidden_states, w1, b1, w2, b2):
    x = np.ascontiguousarray(np.asarray(hidden_states, np.float32)).reshape(-1, H)
    w1 = np.asarray(w1, np.float32)
    b1 = np.asarray(b1, np.float32)
    w2 = np.asarray(w2, np.float32)
    b2 = np.asarray(b2, np.float32)

    # weights in PE-friendly layouts (pure host-side input prep)
    import ml_dtypes

    w1_prep = np.ascontiguousarray(
        w1.reshape(16, 128, 16, 128).transpose(0, 3, 2, 1)
    )  # [ot, p(=h%128), i(=h//128), oi]
    if MM1_MODE == "bf16x3":
        w1_hi = w1_prep.astype(ml_dtypes.bfloat16)
        w1_lo = (w1_prep - w1_hi.astype(np.float32)).astype(ml_dtypes.bfloat16)
    w2_prep = np.ascontiguousarray(w2.reshape(E, 16, 128).transpose(2, 1, 0))
    b1_prep = np.ascontiguousarray(b1.reshape(16, 128).T)
    b2_prep = np.ascontiguousarray(np.tile(b2[None, :], (128, 1)))

    in_maps = []
    for c in range(NCORES):
        xs = x[c * TL : (c + 1) * TL]  # [512, 2048]
        x_prep = np.ascontiguousarray(xs.reshape(TL, 16, 128).transpose(2, 1, 0))
        maska = (np.arange(E) > c).astype(np.float32).reshape(E, 1)
        sel = np.zeros((128, 1), np.float32)
        sel[c, 0] = 1.0
        if MM1_MODE == "bf16x3":
            x_hi = x_prep.astype(ml_dtypes.bfloat16)
            x_lo = (x_prep - x_hi.astype(np.float32)).astype(ml_dtypes.bfloat16)
            mm1_inputs = {"x_hi": x_hi, "x_lo": x_lo, "w1_hi": w1_hi, "w1_lo": w1_lo}
        else:
            mm1_inputs = {"x": x_prep, "w1": w1_prep}
        in_maps.append(
            {
                **mm1_inputs,
                "w2": w2_prep,
                "b1": b1_prep,
                "b2": b2_prep,
                "maska": np.ascontiguousarray(maska),
                "sel": sel,
            }
        )
    return in_maps


def _assemble(results):
    disp = np.empty((NCORES * TL, E, CAP), np.float32)
    comb = np.empty((NCORES * TL, E, CAP), np.float32)
    probs = np.empty((NCORES * TL, E), np.float32)
    for c, r in enumerate(results):
        dc = np.asarray(r["dc"]).reshape(TL, E, CAP, 2)
        disp[c * TL : (c + 1) * TL] = dc[..., 0]
        comb[c * TL : (c + 1) * TL] = dc[..., 1]
        probs[c * TL : (c + 1) * TL] = np.asarray(r["probs"])
    aux = np.float32(np.asarray(results[0]["aux"]).reshape(-1)[0])
    return (
        disp.reshape(2, 2048, E, CAP),
        comb.reshape(2, 2048, E, CAP),
        probs.reshape(2, 2048, E),
        aux,
    )


def run(trace=False, **inputs):
    """Run on the 8 NeuronCores; returns (outputs_tuple, exec_time_ns|None)."""
    from concourse.bass_utils import run_bass_kernel_spmd

    nc = _get_nc()
    in_maps = _prep_in_maps(
        inputs["hidden_states"], inputs["w1"], inputs["b1"], inputs["w2"], inputs["b2"]
    )
    res = run_bass_kernel_spmd(nc, in_maps, core_ids=list(range(NCORES)), trace=trace)
    return _assemble(res.results), res.exec_time_ns


def kernel(**inputs):
    outputs, _ = run(trace=False, **inputs)
    return outputs


# revision 7
# speedup vs baseline: 1.3658x; 1.1014x over previous
"""MoE BaseRouter (router MLP + top-2 dispatch/combine) on 8 TRN2 NeuronCores.

Strategy (data-parallel over tokens, per sharding hint):
  - 4096 tokens sharded 512/core. Each core runs the router MLP
    (x @ w1.T -> relu -> @ w2.T) in fp32 (float32r PE mode, full rate),
    softmax + top-2 via the DVE max8/max_index instructions.
  - Capacity-slot positions: per-core inclusive cumsum over local tokens via
    an upper-triangular matmul; cross-core exclusive per-expert offsets via a
    tiny (320 B) AllReduce of masked per-core expert totals. Expert usage /
    router-prob sums for the aux loss ride in the same AllReduce.
  - dispatch/combine are huge (2 x 201 MB) but ~0.02% nonzero. ExternalOutput
    buffers are pre-zeroed by run_bass_kernel_spmd (documented contract), so
    each core scatter-writes only its 1024 (dispatch,combine) pairs via
    indirect DMA; capacity-overflow entries are suppressed with an
    out-of-bounds sentinel index + bounds_check.

kernel(**inputs) takes the FULL inputs and returns the full
(dispatch, combine, router_probs, aux_loss) tuple, matching reference().
"""

import sys

if "/opt/trn_rl_repo" not in sys.path:
    sys.path.insert(0, "/opt/trn_rl_repo")

import numpy as np

H = 2048          # hidden
E = 8             # experts
K = 2             # top-k
CAP = 1536        # capacity = int(4096 * 1.5 * 2 / 8)
NCORES = 8
TL = 512          # tokens per core
NCH = 4           # token chunks of 128 per core
NFLAT = TL * E * CAP  # flat (token, expert, cap) slots per core
OOB = 2e7         # sentinel added to invalid (pos >= CAP) indices
AUX_SCALE = float(E) / (4096.0 * 4096.0 * K)  # == 2**-22
MM1_MODE = "bf16x3"  # "fp32" (exact, 4 cyc/row) | "bf16x3" (hi*hi+hi*lo+lo*hi, 3 cyc/row)

_CACHE = {}


def _build_nc():
    import concourse.mybir as mybir
    from concourse import bacc
    from concourse.bass import IndirectOffsetOnAxis
    from concourse.masks import make_upper_triangular
    from concourse.tile import TileContext

    f32 = mybir.dt.float32
    i32 = mybir.dt.int32
    u32 = mybir.dt.uint32
    Alu = mybir.AluOpType
    Act = mybir.ActivationFunctionType
    AX = mybir.AxisListType

    nc = bacc.Bacc(None, target_bir_lowering=False, debug=False)

    bf16 = mybir.dt.bfloat16
    if MM1_MODE == "fp32":
        x_ins = [nc.declare_dram_parameter("x", [128, 16, TL], f32, isOutput=False)]
        w1_ins = [nc.declare_dram_parameter("w1", [16, 128, 16, 128], f32, isOutput=False)]
    else:
        x_ins = [
            nc.declare_dram_parameter("x_hi", [128, 16, TL], bf16, isOutput=False),
            nc.declare_dram_parameter("x_lo", [128, 16, TL], bf16, isOutput=False),
        ]
        w1_ins = [
            nc.declare_dram_parameter("w1_hi", [16, 128, 16, 128], bf16, isOutput=False),
            nc.declare_dram_parameter("w1_lo", [16, 128, 16, 128], bf16, isOutput=False),
        ]
    w2_in = nc.declare_dram_parameter("w2", [128, 16, E], f32, isOutput=False)
    b1_in = nc.declare_dram_parameter("b1", [128, 16], f32, isOutput=False)
    b2_in = nc.declare_dram_parameter("b2", [128, E], f32, isOutput=False)
    maska_in = nc.declare_dram_parameter("maska", [E, 1], f32, isOutput=False)
    sel_in = nc.declare_dram_parameter("sel", [128, 1], f32, isOutput=False)

    dc_out = nc.declare_dram_parameter("dc", [NFLAT, 2], f32, isOutput=True)
    probs_out = nc.declare_dram_parameter("probs", [TL, E], f32, isOutput=True)
    aux_out = nc.declare_dram_parameter("aux", [1, 1], f32, isOutput=True)

    with TileContext(nc) as tc:
        with (
            tc.tile_pool(name="const", bufs=1) as cpool,
            tc.tile_pool(name="big", bufs=1) as bpool,
            tc.tile_pool(name="w1s", bufs=3) as wpool,
            tc.tile_pool(name="small", bufs=2) as spool,
            tc.tile_pool(name="psum", bufs=1, space="PSUM") as ppool,
            tc.tile_pool(name="dram", bufs=1, space="DRAM") as dpool,
        ):
            # ---------------- constants ----------------
            iota_i = cpool.tile([128, E], i32)
            nc.gpsimd.iota(iota_i[:], pattern=[[1, E]], base=0, channel_multiplier=0)
            iota_f = cpool.tile([128, E], f32)
            nc.vector.tensor_copy(iota_f[:], iota_i[:])

            tokb_i = cpool.tile([128, NCH], i32)
            for j in range(NCH):
                # flat slot base of token (j*128 + p): (j*128 + p) * E*CAP
                nc.gpsimd.iota(
                    tokb_i[:, j : j + 1],
                    pattern=[[1, 1]],
                    base=j * 128 * E * CAP,
                    channel_multiplier=E * CAP,
                )
            tokb_f = cpool.tile([128, NCH], f32)
            nc.vector.tensor_copy(tokb_f[:], tokb_i[:])

            u_incl = cpool.tile([128, 128], f32)
            make_upper_triangular(nc, u_incl[:], val=1.0, diag=True)
            ones128 = cpool.tile([128, 128], f32)
            nc.vector.memset(ones128[:], 1.0)

            # warm up the collectives firmware early (overlaps the matmuls)
            warm_sb = cpool.tile([1, 8], f32)
            nc.vector.memset(warm_sb[:], 1.0)
            warm_in = dpool.tile([8], f32)
            warm_out = dpool.tile([8], f32, addr_space="Shared")
            nc.sync.dma_start(
                out=warm_in[0:8].rearrange("(a e) -> a e", a=1), in_=warm_sb[:]
            )
            nc.gpsimd.collective_compute(
                "AllReduce",
                Alu.add,
                ins=[warm_in[:].opt()],
                outs=[warm_out[:].opt()],
                replica_groups=[list(range(NCORES))],
            )

            # ---------------- inputs to SBUF ----------------
            # first w1 slab loads first so the PE can start ASAP; x chunk 0
            # next; the rest stream behind.
            x_dt = f32 if MM1_MODE == "fp32" else bf16
            w1t0s = []
            for wi, w1_in in enumerate(w1_ins):
                w1t = wpool.tile([128, 16, 128], x_dt, tag=f"w1t{wi}", name=f"w1t{wi}_0")
                nc.sync.dma_start(out=w1t[:], in_=w1_in[0])
                w1t0s.append(w1t)
            x_sbs = []
            for xi, x_in in enumerate(x_ins):
                x_sb = bpool.tile([128, 16, TL], x_dt, name=f"x_sb{xi}")
                x_sbs.append(x_sb)
            for q in range(8):
                for xi, x_in in enumerate(x_ins):
                    nc.sync.dma_start(
                        out=x_sbs[xi][:, 2 * q : 2 * q + 2, :],
                        in_=x_in[:, 2 * q : 2 * q + 2, :],
                    )
            w2_sb = cpool.tile([128, 16, E], f32)
            nc.sync.dma_start(out=w2_sb[:], in_=w2_in[:])
            b1_sb = cpool.tile([128, 16], f32)
            nc.sync.dma_start(out=b1_sb[:], in_=b1_in[:])
            b2_sb = cpool.tile([128, E], f32)
            nc.sync.dma_start(out=b2_sb[:], in_=b2_in[:])
            maska_sb = cpool.tile([E, 1], f32)
            nc.sync.dma_start(out=maska_sb[:], in_=maska_in[:])
            sel_sb = cpool.tile([128, 1], f32)
            nc.sync.dma_start(out=sel_sb[:], in_=sel_in[:])

            # ---------------- phase 1: router MLP ----------------
            h_sb = bpool.tile([128, 16, TL], f32)  # h.T tiles: [o(part), ot, t]
            psum2 = [
                ppool.tile([128, E], f32, tag="mm2", bufs=NCH, name=f"psum2_{j}")
                for j in range(NCH)
            ]
            for ot in range(16):
                if ot == 0:
                    w1ts = w1t0s
                else:
                    w1ts = []
                    for wi, w1_in in enumerate(w1_ins):
                        w1t = wpool.tile(
                            [128, 16, 128], x_dt, tag=f"w1t{wi}", name=f"w1t{wi}_{ot}"
                        )
                        nc.sync.dma_start(out=w1t[:], in_=w1_in[ot])
                        w1ts.append(w1t)
                ps = ppool.tile([128, TL], f32, tag="mm1", bufs=2, name=f"ps1_{ot}")
                if MM1_MODE == "fp32":
                    terms = [(w1ts[0], x_sbs[0])] * 16
                    for i in range(16):
                        nc.tensor.matmul(
                            ps[:],
                            lhsT=w1ts[0][:, i, :],
                            rhs=x_sbs[0][:, i, :],
                            start=(i == 0),
                            stop=(i == 15),
                        )
                else:
                    # h = x_hi@w_hi + x_hi@w_lo + x_lo@w_hi (lo*lo dropped)
                    n_mm = 0
                    for i in range(16):
                        for wt, xt in (
                            (w1ts[0], x_sbs[0]),
                            (w1ts[1], x_sbs[0]),
                            (w1ts[0], x_sbs[1]),
                        ):
                            nc.tensor.matmul(
                                ps[:],
                                lhsT=wt[:, i, :],
                                rhs=xt[:, i, :],
                                start=(n_mm == 0),
                                stop=(n_mm == 47),
                            )
                            n_mm += 1
                nc.scalar.activation(
                    out=h_sb[:, ot, :],
                    in_=ps[:],
                    func=Act.Relu,
                    bias=b1_sb[:, ot : ot + 1],
                    scale=1.0,
                )
                # interleaved second matmul: logits[t, e] += h[t, hdim]*w2[e, hdim]
                for j in range(NCH):
                    nc.tensor.matmul(
                        psum2[j][:],
                        lhsT=h_sb[:, ot, j * 128 : (j + 1) * 128],
                        rhs=w2_sb[:, ot, :],
                        start=(ot == 0),
                        stop=(ot == 15),
                        skip_group_check=True,
                    )

            # ---------------- phase 2: per-chunk routing ----------------
            probs_all = bpool.tile([128, NCH, E], f32)
            eq0_all = bpool.tile([128, NCH, E], f32)
            eq1_all = bpool.tile([128, NCH, E], f32)
            cbef_all = bpool.tile([128, NCH, E], f32)
            if_all = bpool.tile([128, NCH, K], f32)
            val2_all = bpool.tile([128, NCH, K, 2], f32)
            nc.vector.memset(val2_all[:, :, :, 0:1], 1.0)  # dispatch value
            acc_sb = bpool.tile([128, E], f32)  # running expert totals (bcast)
            spr_sb = bpool.tile([1, E], f32)  # running sum of probs
            poskl_all = bpool.tile([128, NCH, K], f32)  # local slot positions
            basek_all = bpool.tile([128, NCH, K], f32)  # tok*E*CAP + e*CAP + poskl

            for j in range(NCH):
                logits = spool.tile([128, E], f32, tag="logits", bufs=2)
                nc.vector.tensor_tensor(
                    out=logits[:], in0=psum2[j][:], in1=b2_sb[:], op=Alu.add
                )
                mx = spool.tile([128, 1], f32, tag="mx", bufs=2)
                nc.vector.reduce_max(out=mx[:], in_=logits[:], axis=AX.X)
                nmx = spool.tile([128, 1], f32, tag="nmx", bufs=2)
                nc.vector.tensor_scalar_mul(nmx[:], mx[:], -1.0)
                pexp = spool.tile([128, E], f32, tag="pexp", bufs=2)
                nc.scalar.activation(
                    out=pexp[:], in_=logits[:], func=Act.Exp, bias=nmx[:, 0:1], scale=1.0
                )
                sm = spool.tile([128, 1], f32, tag="sm", bufs=2)
                nc.vector.reduce_sum(out=sm[:], in_=pexp[:], axis=AX.X)
                rsm = spool.tile([128, 1], f32, tag="rsm", bufs=2)
                nc.vector.reciprocal(rsm[:], sm[:])
                nc.vector.tensor_scalar_mul(probs_all[:, j, :], pexp[:], rsm[:, 0:1])
                nc.sync.dma_start(
                    out=probs_out[j * 128 : (j + 1) * 128, :], in_=probs_all[:, j, :]
                )

                top8 = spool.tile([128, 8], f32, tag="top8", bufs=2)
                nc.vector.max(out=top8[:], in_=probs_all[:, j, :])
                idx8 = spool.tile([128, 8], u32, tag="idx8", bufs=2)
                nc.vector.max_index(out=idx8[:], in_max=top8[:], in_values=probs_all[:, j, :])

                # renormalized top-2 probs -> combine values
                den = spool.tile([128, 1], f32, tag="den", bufs=2)
                nc.vector.scalar_tensor_tensor(
                    out=den[:], in0=top8[:, 0:1], scalar=1e-8,
                    in1=top8[:, 1:2], op0=Alu.add, op1=Alu.add,
                )
                rden = spool.tile([128, 1], f32, tag="rden", bufs=2)
                nc.vector.reciprocal(rden[:], den[:])
                nc.vector.tensor_tensor(
                    out=val2_all[:, j, 0, 1:2], in0=top8[:, 0:1], in1=rden[:], op=Alu.mult
                )
                nc.vector.tensor_tensor(
                    out=val2_all[:, j, 1, 1:2], in0=top8[:, 1:2], in1=rden[:], op=Alu.mult
                )

                # expert ids as f32 + one-hots
                nc.vector.tensor_copy(if_all[:, j, 0:1], idx8[:, 0:1])
                nc.vector.tensor_copy(if_all[:, j, 1:2], idx8[:, 1:2])
                nc.vector.tensor_tensor(
                    out=eq0_all[:, j, :], in0=iota_f[:],
                    in1=if_all[:, j, 0:1].to_broadcast([128, E]), op=Alu.is_equal,
                )
                nc.vector.tensor_tensor(
                    out=eq1_all[:, j, :], in0=iota_f[:],
                    in1=if_all[:, j, 1:2].to_broadcast([128, E]), op=Alu.is_equal,
                )
                oh = spool.tile([128, E], f32, tag="oh", bufs=2)
                nc.vector.tensor_tensor(
                    out=oh[:], in0=eq0_all[:, j, :], in1=eq1_all[:, j, :], op=Alu.add
                )

                # local (pre-offset) positions and flat index bases for both
                # top-k slots: everything that does not need the AllReduce.
                # (filled below once cbef is ready)
                # local inclusive cumsum + chunk totals (broadcast to 128 parts)
                cntu = ppool.tile([128, E], f32, tag="auxp", bufs=2, name=f"cntu_{j}")
                nc.tensor.matmul(cntu[:], lhsT=u_incl[:], rhs=oh[:], start=True, stop=True)
                totb = ppool.tile([128, E], f32, tag="auxp", bufs=2, name=f"totb_{j}")
                nc.tensor.matmul(totb[:], lhsT=ones128[:], rhs=oh[:], start=True, stop=True)
                sprow = ppool.tile([1, E], f32, tag="auxp", bufs=2, name=f"sprow_{j}")
                nc.tensor.matmul(
                    sprow[:], lhsT=ones128[:, 0:1], rhs=probs_all[:, j, :],
                    start=True, stop=True,
                )

                # cnt_before = (inclusive - own) + totals of earlier chunks
                nc.vector.tensor_tensor(
                    out=cbef_all[:, j, :], in0=cntu[:], in1=oh[:], op=Alu.subtract
                )
                if j > 0:
                    nc.vector.tensor_tensor(
                        out=cbef_all[:, j, :], in0=cbef_all[:, j, :], in1=acc_sb[:],
                        op=Alu.add,
                    )
                    nc.vector.tensor_tensor(
                        out=acc_sb[:], in0=acc_sb[:], in1=totb[:], op=Alu.add
                    )
                    nc.vector.tensor_tensor(
                        out=spr_sb[:], in0=spr_sb[:], in1=sprow[:], op=Alu.add
                    )
                else:
                    nc.vector.tensor_copy(acc_sb[:], totb[:])
                    nc.vector.tensor_copy(spr_sb[:], sprow[:])

                for k, eqk in ((0, eq0_all), (1, eq1_all)):
                    tmp8 = spool.tile([128, E], f32, tag="tmp8", bufs=2)
                    nc.vector.scalar_tensor_tensor(
                        out=tmp8[:], in0=cbef_all[:, j, :], scalar=0.0,
                        in1=eqk[:, j, :], op0=Alu.add, op1=Alu.mult,
                        accum_out=poskl_all[:, j, k : k + 1],
                    )
                    nc.vector.scalar_tensor_tensor(
                        out=basek_all[:, j, k : k + 1],
                        in0=if_all[:, j, k : k + 1], scalar=float(CAP),
                        in1=tokb_f[:, j : j + 1], op0=Alu.mult, op1=Alu.add,
                    )
                    nc.vector.tensor_tensor(
                        out=basek_all[:, j, k : k + 1],
                        in0=basek_all[:, j, k : k + 1],
                        in1=poskl_all[:, j, k : k + 1], op=Alu.add,
                    )

            # ---------------- phase 3: tiny AllReduce ----------------
            contrib = spool.tile([E, E], f32)  # [dst_core, expert]
            nc.vector.tensor_scalar_mul(contrib[:], acc_sb[0:E, :], maska_sb[:, 0:1])

            ar_in = dpool.tile([80], f32)
            ar_out = dpool.tile([80], f32, addr_space="Shared")
            nc.sync.dma_start(
                out=ar_in[0:64].rearrange("(c e) -> c e", e=E), in_=contrib[:]
            )
            nc.sync.dma_start(
                out=ar_in[64:72].rearrange("(a e) -> a e", a=1), in_=acc_sb[0:1, :]
            )
            nc.sync.dma_start(
                out=ar_in[72:80].rearrange("(a e) -> a e", a=1), in_=spr_sb[:]
            )
            nc.gpsimd.collective_compute(
                "AllReduce",
                Alu.add,
                ins=[ar_in[:].opt()],
                outs=[ar_out[:].opt()],
                replica_groups=[list(range(NCORES))],
            )

            # ---------------- phase 4: offsets + aux ----------------
            a128 = spool.tile([128, E], f32)
            nc.vector.memset(a128[:], 0.0)
            nc.sync.dma_start(
                out=a128[0:E, :], in_=ar_out[0:64].rearrange("(c e) -> c e", e=E)
            )
            g_sb = spool.tile([1, 2 * E], f32)
            nc.sync.dma_start(
                out=g_sb[:], in_=ar_out[64:80].rearrange("(a x) -> a x", a=1)
            )

            selmat = spool.tile([128, 128], f32)
            nc.vector.tensor_scalar_mul(selmat[:], ones128[:], sel_sb[:, 0:1])
            offs = ppool.tile([128, E], f32, tag="auxp", bufs=2)
            nc.tensor.matmul(offs[:], lhsT=selmat[:], rhs=a128[:], start=True, stop=True)

            auxv = spool.tile([1, E], f32)
            nc.vector.tensor_tensor(
                out=auxv[:], in0=g_sb[:, 0:E], in1=g_sb[:, E : 2 * E], op=Alu.mult
            )
            auxs = spool.tile([1, 1], f32)
            nc.vector.reduce_sum(out=auxs[:], in_=auxv[:], axis=AX.X)
            nc.vector.tensor_scalar_mul(auxs[:], auxs[:], AUX_SCALE)
            nc.sync.dma_start(out=aux_out[:], in_=auxs[:])

            # ---------------- phase 5: positions + scatter ----------------
            # post-AR: per-token expert offset, overflow suppression, and ONE
            # merged indirect scatter of all (chunk, k) slots.
            idx_i = bpool.tile([128, NCH, K], i32)
            for k, eqk in ((0, eq0_all), (1, eq1_all)):
                offm = spool.tile([128, NCH, E], f32, tag="offm", bufs=2)
                nc.vector.tensor_tensor(
                    out=offm[:],
                    in0=eqk[:, :, :],
                    in1=offs[:].unsqueeze(1).to_broadcast([128, NCH, E]),
                    op=Alu.mult,
                )
                offk = spool.tile([128, NCH], f32, tag="offk", bufs=2)
                nc.vector.reduce_sum(out=offk[:], in_=offm[:], axis=AX.X)
                posv = spool.tile([128, NCH], f32, tag="posv", bufs=2)
                nc.vector.tensor_tensor(
                    out=posv[:], in0=poskl_all[:, :, k], in1=offk[:], op=Alu.add
                )
                over = spool.tile([128, NCH], f32, tag="over", bufs=2)
                nc.vector.tensor_scalar(
                    out=over[:], in0=posv[:], scalar1=CAP - 0.5, scalar2=None,
                    op0=Alu.is_gt,
                )
                idxf = spool.tile([128, NCH], f32, tag="idxf", bufs=2)
                nc.vector.tensor_tensor(
                    out=idxf[:], in0=basek_all[:, :, k], in1=offk[:], op=Alu.add
                )
                nc.vector.scalar_tensor_tensor(
                    out=idxf[:], in0=over[:], scalar=OOB, in1=idxf[:],
                    op0=Alu.mult, op1=Alu.add,
                )
                nc.vector.tensor_copy(idx_i[:, :, k], idxf[:])
            # one indirect scatter per (chunk, k): multi-offset-per-partition
            # indirect DMA is NOT honored by hardware (sim-only semantics).
            for j in range(NCH):
                for k in range(K):
                    nc.gpsimd.indirect_dma_start(
                        out=dc_out[:, :],
                        out_offset=IndirectOffsetOnAxis(
                            ap=idx_i[:, j, k : k + 1], axis=0
                        ),
                        in_=val2_all[:, j, k, :],
                        in_offset=None,
                        bounds_check=NFLAT - 1,
                        oob_is_err=False,
                    )

    nc.finalize()  # run bacc passes (register allocation etc.)
    return nc


def _get_nc():
    if "nc" not in _CACHE:
        _CACHE["nc"] = _build_nc()
    return _CACHE["nc"]


def _prep_in_maps(hidden_states, w1, b1, w2, b2):
    x = np.ascontiguousarray(np.asarray(hidden_states, np.float32)).reshape(-1, H)
    w1 = np.asarray(w1, np.float32)
    b1 = np.asarray(b1, np.float32)
    w2 = np.asarray(w2, np.float32)
    b2 = np.asarray(b2, np.float32)

    # weights in PE-friendly layouts (pure host-side input prep)
    import ml_dtypes

    w1_prep = np.ascontiguousarray(
        w1.reshape(16, 128, 16, 128).transpose(0, 3, 2, 1)
    )  # [ot, p(=h%128), i(=h//128), oi]
    if MM1_MODE == "bf16x3":
        w1_hi = w1_prep.astype(ml_dtypes.bfloat16)
        w1_lo = (w1_prep - w1_hi.astype(np.float32)).astype(ml_dtypes.bfloat16)
    w2_prep = np.ascontiguousarray(w2.reshape(E, 16, 128).transpose(2, 1, 0))
    b1_prep = np.ascontiguousarray(b1.reshape(16, 128).T)
    b2_prep = np.ascontiguousarray(np.tile(b2[None, :], (128, 1)))

    in_maps = []
    for c in range(NCORES):
        xs = x[c * TL : (c + 1) * TL]  # [512, 2048]
        x_prep = np.ascontiguousarray(xs.reshape(TL, 16, 128).transpose(2, 1, 0))
        maska = (np.arange(E) > c).astype(np.float32).reshape(E, 1)
        sel = np.zeros((128, 1), np.float32)
        sel[c, 0] = 1.0
        if MM1_MODE == "bf16x3":
            x_hi = x_prep.astype(ml_dtypes.bfloat16)
            x_lo = (x_prep - x_hi.astype(np.float32)).astype(ml_dtypes.bfloat16)
            mm1_inputs = {"x_hi": x_hi, "x_lo": x_lo, "w1_hi": w1_hi, "w1_lo": w1_lo}
        else:
            mm1_inputs = {"x": x_prep, "w1": w1_prep}
        in_maps.append(
            {
                **mm1_inputs,
                "w2": w2_prep,
                "b1": b1_prep,
                "b2": b2_prep,
                "maska": np.ascontiguousarray(maska),
                "sel": sel,
            }
        )
    return in_maps


def _assemble(results):
    disp = np.empty((NCORES * TL, E, CAP), np.float32)
    comb = np.empty((NCORES * TL, E, CAP), np.float32)
    probs = np.empty((NCORES * TL, E), np.float32)
    for c, r in enumerate(results):
        dc = np.asarray(r["dc"]).reshape(TL, E, CAP, 2)
        disp[c * TL : (c + 1) * TL] = dc[..., 0]
        comb[c * TL : (c + 1) * TL] = dc[..., 1]
        probs[c * TL : (c + 1) * TL] = np.asarray(r["probs"])
    aux = np.float32(np.asarray(results[0]["aux"]).reshape(-1)[0])
    return (
        disp.reshape(2, 2048, E, CAP),
        comb.reshape(2, 2048, E, CAP),
        probs.reshape(2, 2048, E),
        aux,
    )


def run(trace=False, **inputs):
    """Run on the 8 NeuronCores; returns (outputs_tuple, exec_time_ns|None)."""
    from concourse.bass_utils import run_bass_kernel_spmd

    nc = _get_nc()
    in_maps = _prep_in_maps(
        inputs["hidden_states"], inputs["w1"], inputs["b1"], inputs["w2"], inputs["b2"]
    )
    res = run_bass_kernel_spmd(nc, in_maps, core_ids=list(range(NCORES)), trace=trace)
    return _assemble(res.results), res.exec_time_ns


def kernel(**inputs):
    outputs, _ = run(trace=False, **inputs)
    return outputs


# revision 10
# speedup vs baseline: 1.4002x; 1.0252x over previous
"""MoE BaseRouter (router MLP + top-2 dispatch/combine) on 8 TRN2 NeuronCores.

Strategy (data-parallel over tokens, per sharding hint):
  - 4096 tokens sharded 512/core. Each core runs the router MLP
    (x @ w1.T -> relu -> @ w2.T) in fp32 (float32r PE mode, full rate),
    softmax + top-2 via the DVE max8/max_index instructions.
  - Capacity-slot positions: per-core inclusive cumsum over local tokens via
    an upper-triangular matmul; cross-core exclusive per-expert offsets via a
    tiny (320 B) AllReduce of masked per-core expert totals. Expert usage /
    router-prob sums for the aux loss ride in the same AllReduce.
  - dispatch/combine are huge (2 x 201 MB) but ~0.02% nonzero. ExternalOutput
    buffers are pre-zeroed by run_bass_kernel_spmd (documented contract), so
    each core scatter-writes only its 1024 (dispatch,combine) pairs via
    indirect DMA; capacity-overflow entries are suppressed with an
    out-of-bounds sentinel index + bounds_check.

kernel(**inputs) takes the FULL inputs and returns the full
(dispatch, combine, router_probs, aux_loss) tuple, matching reference().
"""

import sys

if "/opt/trn_rl_repo" not in sys.path:
    sys.path.insert(0, "/opt/trn_rl_repo")

import numpy as np

H = 2048          # hidden
E = 8             # experts
K = 2             # top-k
CAP = 1536        # capacity = int(4096 * 1.5 * 2 / 8)
NCORES = 8
TL = 512          # tokens per core
NCH = 4           # token chunks of 128 per core
NFLAT = TL * E * CAP  # flat (token, expert, cap) slots per core
OOB = 2e7         # sentinel added to invalid (pos >= CAP) indices
AUX_SCALE = float(E) / (4096.0 * 4096.0 * K)  # == 2**-22
MM1_MODE = "bf16x3"  # "fp32" (exact, 4 cyc/row) | "bf16x3" (hi*hi+hi*lo+lo*hi, 3 cyc/row)

_CACHE = {}


def _build_nc():
    import concourse.mybir as mybir
    from concourse import bacc
    from concourse.bass import IndirectOffsetOnAxis
    from concourse.masks import make_upper_triangular
    from concourse.tile import TileContext

    f32 = mybir.dt.float32
    i32 = mybir.dt.int32
    u32 = mybir.dt.uint32
    Alu = mybir.AluOpType
    Act = mybir.ActivationFunctionType
    AX = mybir.AxisListType

    nc = bacc.Bacc(None, target_bir_lowering=False, debug=False)

    bf16 = mybir.dt.bfloat16
    if MM1_MODE == "fp32":
        x_ins = [nc.declare_dram_parameter("x", [128, 16, TL], f32, isOutput=False)]
        w1_ins = [nc.declare_dram_parameter("w1", [16, 128, 16, 128], f32, isOutput=False)]
    else:
        x_ins = [
            nc.declare_dram_parameter("x_hi", [128, 16, TL], bf16, isOutput=False),
            nc.declare_dram_parameter("x_lo", [128, 16, TL], bf16, isOutput=False),
        ]
        w1_ins = [
            nc.declare_dram_parameter("w1_hi", [16, 128, 16, 128], bf16, isOutput=False),
            nc.declare_dram_parameter("w1_lo", [16, 128, 16, 128], bf16, isOutput=False),
        ]
    w2_in = nc.declare_dram_parameter("w2", [128, 16, E], f32, isOutput=False)
    b1_in = nc.declare_dram_parameter("b1", [128, 16], f32, isOutput=False)
    b2_in = nc.declare_dram_parameter("b2", [128, E], f32, isOutput=False)
    maska_in = nc.declare_dram_parameter("maska", [E, 1], f32, isOutput=False)
    sel_in = nc.declare_dram_parameter("sel", [128, 1], f32, isOutput=False)

    NBLK = 128 * E * CAP
    dc_outs = [
        nc.declare_dram_parameter(f"dc{j}", [NBLK, 2], f32, isOutput=True)
        for j in range(NCH)
    ]
    probs_out = nc.declare_dram_parameter("probs", [TL, E], f32, isOutput=True)
    aux_out = nc.declare_dram_parameter("aux", [1, 1], f32, isOutput=True)

    with TileContext(nc) as tc:
        with (
            tc.tile_pool(name="const", bufs=1) as cpool,
            tc.tile_pool(name="big", bufs=1) as bpool,
            tc.tile_pool(name="w1s", bufs=3) as wpool,
            tc.tile_pool(name="small", bufs=2) as spool,
            tc.tile_pool(name="psum", bufs=1, space="PSUM") as ppool,
            tc.tile_pool(name="dram", bufs=1, space="DRAM") as dpool,
        ):
            # ---------------- constants ----------------
            iota_i = cpool.tile([128, E], i32)
            nc.gpsimd.iota(iota_i[:], pattern=[[1, E]], base=0, channel_multiplier=0)
            iota_f = cpool.tile([128, E], f32)
            nc.vector.tensor_copy(iota_f[:], iota_i[:])

            tokb_i = cpool.tile([128, 1], i32)
            # flat slot base of token p within its 128-token block: p * E*CAP
            nc.gpsimd.iota(
                tokb_i[:], pattern=[[1, 1]], base=0, channel_multiplier=E * CAP
            )
            tokb_f = cpool.tile([128, 1], f32)
            nc.vector.tensor_copy(tokb_f[:], tokb_i[:])

            u_incl = cpool.tile([128, 128], f32)
            make_upper_triangular(nc, u_incl[:], val=1.0, diag=True)
            ones128 = cpool.tile([128, 128], f32)
            nc.vector.memset(ones128[:], 1.0)

            # ---------------- inputs to SBUF ----------------
            # DMA priority: w1_hi slab 0 + first x_hi chunk feed the first
            # matmul; everything else streams behind.
            x_dt = f32 if MM1_MODE == "fp32" else bf16
            w1t0s = []
            x_sbs = []
            for xi, x_in in enumerate(x_ins):
                x_sb = bpool.tile([128, 16, TL], x_dt, name=f"x_sb{xi}")
                x_sbs.append(x_sb)
            w1t_first = wpool.tile(
                [128, 16, 128], x_dt, tag="w1t0", name="w1t0_0"
            )
            nc.sync.dma_start(out=w1t_first[:], in_=w1_ins[0][0])
            w1t0s.append(w1t_first)
            for q in range(8):
                nc.sync.dma_start(
                    out=x_sbs[0][:, 2 * q : 2 * q + 2, :],
                    in_=x_ins[0][:, 2 * q : 2 * q + 2, :],
                )
            if len(x_ins) > 1:
                for q in range(8):
                    nc.sync.dma_start(
                        out=x_sbs[1][:, 2 * q : 2 * q + 2, :],
                        in_=x_ins[1][:, 2 * q : 2 * q + 2, :],
                    )
                w1t_first_lo = wpool.tile(
                    [128, 16, 128], x_dt, tag="w1t1", name="w1t1_0"
                )
                nc.sync.dma_start(out=w1t_first_lo[:], in_=w1_ins[1][0])
                w1t0s.append(w1t_first_lo)

            # warm up the collectives firmware early (overlaps the matmuls)
            warm_sb = cpool.tile([1, 8], f32)
            nc.vector.memset(warm_sb[:], 1.0)
            warm_in = dpool.tile([8], f32)
            warm_out = dpool.tile([8], f32, addr_space="Shared")
            nc.sync.dma_start(
                out=warm_in[0:8].rearrange("(a e) -> a e", a=1), in_=warm_sb[:]
            )
            nc.gpsimd.collective_compute(
                "AllReduce",
                Alu.add,
                ins=[warm_in[:].opt()],
                outs=[warm_out[:].opt()],
                replica_groups=[list(range(NCORES))],
            )
            w2_sb = cpool.tile([128, 16, E], f32)
            nc.sync.dma_start(out=w2_sb[:], in_=w2_in[:])
            b1_sb = cpool.tile([128, 16], f32)
            nc.sync.dma_start(out=b1_sb[:], in_=b1_in[:])
            b2_sb = cpool.tile([128, E], f32)
            nc.sync.dma_start(out=b2_sb[:], in_=b2_in[:])
            maska_sb = cpool.tile([E, 1], f32)
            nc.sync.dma_start(out=maska_sb[:], in_=maska_in[:])
            sel_sb = cpool.tile([128, 1], f32)
            nc.sync.dma_start(out=sel_sb[:], in_=sel_in[:])

            # ---------------- phase 1: router MLP ----------------
            h_sb = bpool.tile([128, 16, TL], f32)  # h.T tiles: [o(part), ot, t]
            psum2 = [
                ppool.tile([128, E], f32, tag="mm2", bufs=NCH, name=f"psum2_{j}")
                for j in range(NCH)
            ]
            for ot in range(16):
                if ot == 0:
                    w1ts = w1t0s
                else:
                    w1ts = []
                    for wi, w1_in in enumerate(w1_ins):
                        w1t = wpool.tile(
                            [128, 16, 128], x_dt, tag=f"w1t{wi}", name=f"w1t{wi}_{ot}"
                        )
                        nc.sync.dma_start(out=w1t[:], in_=w1_in[ot])
                        w1ts.append(w1t)
                ps = ppool.tile([128, TL], f32, tag="mm1", bufs=2, name=f"ps1_{ot}")
                if MM1_MODE == "fp32":
                    terms = [(w1ts[0], x_sbs[0])] * 16
                    for i in range(16):
                        nc.tensor.matmul(
                            ps[:],
                            lhsT=w1ts[0][:, i, :],
                            rhs=x_sbs[0][:, i, :],
                            start=(i == 0),
                            stop=(i == 15),
                        )
                else:
                    # h = x_hi@w_hi + x_lo@w_hi + x_hi@w_lo (lo*lo dropped);
                    # main pass first so the first matmul only needs the hi
                    # slab + first x_hi chunk.
                    n_mm = 0
                    for wt, xt in (
                        (w1ts[0], x_sbs[0]),
                        (w1ts[0], x_sbs[1]),
                        (w1ts[1], x_sbs[0]),
                    ):
                        for i in range(16):
                            nc.tensor.matmul(
                                ps[:],
                                lhsT=wt[:, i, :],
                                rhs=xt[:, i, :],
                                start=(n_mm == 0),
                                stop=(n_mm == 47),
                            )
                            n_mm += 1
                nc.scalar.activation(
                    out=h_sb[:, ot, :],
                    in_=ps[:],
                    func=Act.Relu,
                    bias=b1_sb[:, ot : ot + 1],
                    scale=1.0,
                )
                # interleaved second matmul: logits[t, e] += h[t, hdim]*w2[e, hdim]
                for j in range(NCH):
                    nc.tensor.matmul(
                        psum2[j][:],
                        lhsT=h_sb[:, ot, j * 128 : (j + 1) * 128],
                        rhs=w2_sb[:, ot, :],
                        start=(ot == 0),
                        stop=(ot == 15),
                        skip_group_check=True,
                    )

            # ---------------- phase 2: per-chunk routing ----------------
            probs_all = bpool.tile([128, NCH, E], f32)
            eq0_all = bpool.tile([128, NCH, E], f32)
            eq1_all = bpool.tile([128, NCH, E], f32)
            cbef_all = bpool.tile([128, NCH, E], f32)
            if_all = bpool.tile([128, NCH, K], f32)
            val2_all = bpool.tile([128, NCH, K, 2], f32)
            nc.vector.memset(val2_all[:, :, :, 0:1], 1.0)  # dispatch value
            acc_sb = bpool.tile([128, E], f32)  # running expert totals (bcast)
            spr_sb = bpool.tile([1, E], f32)  # running sum of probs
            poskl_all = bpool.tile([128, NCH, K], f32)  # local slot positions
            basek_all = bpool.tile([128, NCH, K], f32)  # tok*E*CAP + e*CAP + poskl

            for j in range(NCH):
                logits = spool.tile([128, E], f32, tag="logits", bufs=2)
                nc.vector.tensor_tensor(
                    out=logits[:], in0=psum2[j][:], in1=b2_sb[:], op=Alu.add
                )
                mx = spool.tile([128, 1], f32, tag="mx", bufs=2)
                nc.vector.reduce_max(out=mx[:], in_=logits[:], axis=AX.X)
                nmx = spool.tile([128, 1], f32, tag="nmx", bufs=2)
                nc.vector.tensor_scalar_mul(nmx[:], mx[:], -1.0)
                pexp = spool.tile([128, E], f32, tag="pexp", bufs=2)
                nc.scalar.activation(
                    out=pexp[:], in_=logits[:], func=Act.Exp, bias=nmx[:, 0:1], scale=1.0
                )
                sm = spool.tile([128, 1], f32, tag="sm", bufs=2)
                nc.vector.reduce_sum(out=sm[:], in_=pexp[:], axis=AX.X)
                rsm = spool.tile([128, 1], f32, tag="rsm", bufs=2)
                nc.vector.reciprocal(rsm[:], sm[:])
                nc.vector.tensor_scalar_mul(probs_all[:, j, :], pexp[:], rsm[:, 0:1])
                nc.sync.dma_start(
                    out=probs_out[j * 128 : (j + 1) * 128, :], in_=probs_all[:, j, :]
                )

                top8 = spool.tile([128, 8], f32, tag="top8", bufs=2)
                nc.vector.max(out=top8[:], in_=probs_all[:, j, :])
                idx8 = spool.tile([128, 8], u32, tag="idx8", bufs=2)
                nc.vector.max_index(out=idx8[:], in_max=top8[:], in_values=probs_all[:, j, :])

                # renormalized top-2 probs -> combine values
                den = spool.tile([128, 1], f32, tag="den", bufs=2)
                nc.vector.scalar_tensor_tensor(
                    out=den[:], in0=top8[:, 0:1], scalar=1e-8,
                    in1=top8[:, 1:2], op0=Alu.add, op1=Alu.add,
                )
                rden = spool.tile([128, 1], f32, tag="rden", bufs=2)
                nc.vector.reciprocal(rden[:], den[:])
                nc.vector.tensor_tensor(
                    out=val2_all[:, j, 0, 1:2], in0=top8[:, 0:1], in1=rden[:], op=Alu.mult
                )
                nc.vector.tensor_tensor(
                    out=val2_all[:, j, 1, 1:2], in0=top8[:, 1:2], in1=rden[:], op=Alu.mult
                )

                # expert ids as f32 + one-hots
                nc.vector.tensor_copy(if_all[:, j, 0:1], idx8[:, 0:1])
                nc.vector.tensor_copy(if_all[:, j, 1:2], idx8[:, 1:2])
                nc.vector.tensor_tensor(
                    out=eq0_all[:, j, :], in0=iota_f[:],
                    in1=if_all[:, j, 0:1].to_broadcast([128, E]), op=Alu.is_equal,
                )
                nc.vector.tensor_tensor(
                    out=eq1_all[:, j, :], in0=iota_f[:],
                    in1=if_all[:, j, 1:2].to_broadcast([128, E]), op=Alu.is_equal,
                )
                oh = spool.tile([128, E], f32, tag="oh", bufs=2)
                nc.vector.tensor_tensor(
                    out=oh[:], in0=eq0_all[:, j, :], in1=eq1_all[:, j, :], op=Alu.add
                )

                # local (pre-offset) positions and flat index bases for both
                # top-k slots: everything that does not need the AllReduce.
                # (filled below once cbef is ready)
                # local inclusive cumsum + chunk totals (broadcast to 128 parts)
                cntu = ppool.tile([128, E], f32, tag="auxp", bufs=2, name=f"cntu_{j}")
                nc.tensor.matmul(cntu[:], lhsT=u_incl[:], rhs=oh[:], start=True, stop=True)
                totb = ppool.tile([128, E], f32, tag="auxp", bufs=2, name=f"totb_{j}")
                nc.tensor.matmul(totb[:], lhsT=ones128[:], rhs=oh[:], start=True, stop=True)
                sprow = ppool.tile([1, E], f32, tag="auxp", bufs=2, name=f"sprow_{j}")
                nc.tensor.matmul(
                    sprow[:], lhsT=ones128[:, 0:1], rhs=probs_all[:, j, :],
                    start=True, stop=True,
                )

                # cnt_before = (inclusive - own) + totals of earlier chunks
                nc.vector.tensor_tensor(
                    out=cbef_all[:, j, :], in0=cntu[:], in1=oh[:], op=Alu.subtract
                )
                if j > 0:
                    nc.vector.tensor_tensor(
                        out=cbef_all[:, j, :], in0=cbef_all[:, j, :], in1=acc_sb[:],
                        op=Alu.add,
                    )
                    nc.vector.tensor_tensor(
                        out=acc_sb[:], in0=acc_sb[:], in1=totb[:], op=Alu.add
                    )
                    nc.vector.tensor_tensor(
                        out=spr_sb[:], in0=spr_sb[:], in1=sprow[:], op=Alu.add
                    )
                else:
                    nc.vector.tensor_copy(acc_sb[:], totb[:])
                    nc.vector.tensor_copy(spr_sb[:], sprow[:])

                for k, eqk in ((0, eq0_all), (1, eq1_all)):
                    tmp8 = spool.tile([128, E], f32, tag="tmp8", bufs=2)
                    nc.vector.scalar_tensor_tensor(
                        out=tmp8[:], in0=cbef_all[:, j, :], scalar=0.0,
                        in1=eqk[:, j, :], op0=Alu.add, op1=Alu.mult,
                        accum_out=poskl_all[:, j, k : k + 1],
                    )
                    nc.vector.scalar_tensor_tensor(
                        out=basek_all[:, j, k : k + 1],
                        in0=if_all[:, j, k : k + 1], scalar=float(CAP),
                        in1=tokb_f[:, 0:1], op0=Alu.mult, op1=Alu.add,
                    )
                    nc.vector.tensor_tensor(
                        out=basek_all[:, j, k : k + 1],
                        in0=basek_all[:, j, k : k + 1],
                        in1=poskl_all[:, j, k : k + 1], op=Alu.add,
                    )

            # ---------------- phase 3: tiny AllReduce ----------------
            contrib = spool.tile([E, E], f32)  # [dst_core, expert]
            nc.vector.tensor_scalar_mul(contrib[:], acc_sb[0:E, :], maska_sb[:, 0:1])

            ar_in = dpool.tile([88], f32)
            ar_out = dpool.tile([88], f32, addr_space="Shared")
            # data-dependency on the warm-up collective: forces ncfw op order
            nc.sync.dma_start(out=ar_in[80:88], in_=warm_out[0:8])
            nc.sync.dma_start(
                out=ar_in[0:64].rearrange("(c e) -> c e", e=E), in_=contrib[:]
            )
            nc.sync.dma_start(
                out=ar_in[64:72].rearrange("(a e) -> a e", a=1), in_=acc_sb[0:1, :]
            )
            nc.sync.dma_start(
                out=ar_in[72:80].rearrange("(a e) -> a e", a=1), in_=spr_sb[:]
            )
            nc.gpsimd.collective_compute(
                "AllReduce",
                Alu.add,
                ins=[ar_in[:].opt()],
                outs=[ar_out[:].opt()],
                replica_groups=[list(range(NCORES))],
            )

            # ---------------- phase 4: offsets + aux ----------------
            a128 = spool.tile([128, E], f32)
            nc.vector.memset(a128[:], 0.0)
            nc.sync.dma_start(
                out=a128[0:E, :], in_=ar_out[0:64].rearrange("(c e) -> c e", e=E)
            )
            g_sb = spool.tile([1, 2 * E], f32)
            nc.sync.dma_start(
                out=g_sb[:], in_=ar_out[64:80].rearrange("(a x) -> a x", a=1)
            )

            selmat = spool.tile([128, 128], f32)
            nc.vector.tensor_scalar_mul(selmat[:], ones128[:], sel_sb[:, 0:1])
            offs = ppool.tile([128, E], f32, tag="auxp", bufs=2)
            nc.tensor.matmul(offs[:], lhsT=selmat[:], rhs=a128[:], start=True, stop=True)

            auxv = spool.tile([1, E], f32)
            nc.vector.tensor_tensor(
                out=auxv[:], in0=g_sb[:, 0:E], in1=g_sb[:, E : 2 * E], op=Alu.mult
            )
            auxs = spool.tile([1, 1], f32)
            nc.vector.reduce_sum(out=auxs[:], in_=auxv[:], axis=AX.X)
            nc.vector.tensor_scalar_mul(auxs[:], auxs[:], AUX_SCALE)
            nc.sync.dma_start(out=aux_out[:], in_=auxs[:])

            # ---------------- phase 5: positions + scatter ----------------
            # post-AR: per-token expert offset, overflow suppression, and ONE
            # merged indirect scatter of all (chunk, k) slots.
            idx_i = bpool.tile([128, NCH, K], i32)
            for k, eqk in ((0, eq0_all), (1, eq1_all)):
                offm = spool.tile([128, NCH, E], f32, tag="offm", bufs=2)
                nc.vector.tensor_tensor(
                    out=offm[:],
                    in0=eqk[:, :, :],
                    in1=offs[:].unsqueeze(1).to_broadcast([128, NCH, E]),
                    op=Alu.mult,
                )
                offk = spool.tile([128, NCH], f32, tag="offk", bufs=2)
                nc.vector.reduce_sum(out=offk[:], in_=offm[:], axis=AX.X)
                posv = spool.tile([128, NCH], f32, tag="posv", bufs=2)
                nc.vector.tensor_tensor(
                    out=posv[:], in0=poskl_all[:, :, k], in1=offk[:], op=Alu.add
                )
                over = spool.tile([128, NCH], f32, tag="over", bufs=2)
                nc.vector.tensor_scalar(
                    out=over[:], in0=posv[:], scalar1=CAP - 0.5, scalar2=None,
                    op0=Alu.is_gt,
                )
                idxf = spool.tile([128, NCH], f32, tag="idxf", bufs=2)
                nc.vector.tensor_tensor(
                    out=idxf[:], in0=basek_all[:, :, k], in1=offk[:], op=Alu.add
                )
                nc.vector.scalar_tensor_tensor(
                    out=idxf[:], in0=over[:], scalar=OOB, in1=idxf[:],
                    op0=Alu.mult, op1=Alu.add,
                )
                nc.vector.tensor_copy(idx_i[:, :, k], idxf[:])
            # one indirect scatter per (chunk, k): multi-offset-per-partition
            # indirect DMA is NOT honored by hardware (sim-only semantics).
            for j in range(NCH):
                for k in range(K):
                    nc.gpsimd.indirect_dma_start(
                        out=dc_outs[j][:, :],
                        out_offset=IndirectOffsetOnAxis(
                            ap=idx_i[:, j, k : k + 1], axis=0
                        ),
                        in_=val2_all[:, j, k, :],
                        in_offset=None,
                        bounds_check=NBLK - 1,
                        oob_is_err=False,
                    )

    nc.finalize()  # run bacc passes (register allocation etc.)
    return nc


def _get_nc():
    if "nc" not in _CACHE:
        _CACHE["nc"] = _build_nc()
    return _CACHE["nc"]


def _prep_in_maps(hidden_states, w1, b1, w2, b2):
    x = np.ascontiguousarray(np.asarray(hidden_states, np.float32)).reshape(-1, H)
    w1 = np.asarray(w1, np.float32)
    b1 = np.asarray(b1, np.float32)
    w2 = np.asarray(w2, np.float32)
    b2 = np.asarray(b2, np.float32)

    # weights in PE-friendly layouts (pure host-side input prep)
    import ml_dtypes

    w1_prep = np.ascontiguousarray(
        w1.reshape(16, 128, 16, 128).transpose(0, 3, 2, 1)
    )  # [ot, p(=h%128), i(=h//128), oi]
    if MM1_MODE == "bf16x3":
        w1_hi = w1_prep.astype(ml_dtypes.bfloat16)
        w1_lo = (w1_prep - w1_hi.astype(np.float32)).astype(ml_dtypes.bfloat16)
    w2_prep = np.ascontiguousarray(w2.reshape(E, 16, 128).transpose(2, 1, 0))
    b1_prep = np.ascontiguousarray(b1.reshape(16, 128).T)
    b2_prep = np.ascontiguousarray(np.tile(b2[None, :], (128, 1)))

    in_maps = []
    for c in range(NCORES):
        xs = x[c * TL : (c + 1) * TL]  # [512, 2048]
        x_prep = np.ascontiguousarray(xs.reshape(TL, 16, 128).transpose(2, 1, 0))
        maska = (np.arange(E) > c).astype(np.float32).reshape(E, 1)
        sel = np.zeros((128, 1), np.float32)
        sel[c, 0] = 1.0
        if MM1_MODE == "bf16x3":
            x_hi = x_prep.astype(ml_dtypes.bfloat16)
            x_lo = (x_prep - x_hi.astype(np.float32)).astype(ml_dtypes.bfloat16)
            mm1_inputs = {"x_hi": x_hi, "x_lo": x_lo, "w1_hi": w1_hi, "w1_lo": w1_lo}
        else:
            mm1_inputs = {"x": x_prep, "w1": w1_prep}
        in_maps.append(
            {
                **mm1_inputs,
                "w2": w2_prep,
                "b1": b1_prep,
                "b2": b2_prep,
                "maska": np.ascontiguousarray(maska),
                "sel": sel,
            }
        )
    return in_maps


def _assemble(results):
    disp = np.empty((NCORES * TL, E, CAP), np.float32)
    comb = np.empty((NCORES * TL, E, CAP), np.float32)
    probs = np.empty((NCORES * TL, E), np.float32)
    for c, r in enumerate(results):
        for j in range(NCH):
            dc = np.asarray(r[f"dc{j}"]).reshape(128, E, CAP, 2)
            t0 = c * TL + j * 128
            disp[t0 : t0 + 128] = dc[..., 0]
            comb[t0 : t0 + 128] = dc[..., 1]
        probs[c * TL : (c + 1) * TL] = np.asarray(r["probs"])
    aux = np.float32(np.asarray(results[0]["aux"]).reshape(-1)[0])
    return (
        disp.reshape(2, 2048, E, CAP),
        comb.reshape(2, 2048, E, CAP),
        probs.reshape(2, 2048, E),
        aux,
    )


def run(trace=False, **inputs):
    """Run on the 8 NeuronCores; returns (outputs_tuple, exec_time_ns|None)."""
    from concourse.bass_utils import run_bass_kernel_spmd

    nc = _get_nc()
    in_maps = _prep_in_maps(
        inputs["hidden_states"], inputs["w1"], inputs["b1"], inputs["w2"], inputs["b2"]
    )
    res = run_bass_kernel_spmd(nc, in_maps, core_ids=list(range(NCORES)), trace=trace)
    return _assemble(res.results), res.exec_time_ns


def kernel(**inputs):
    outputs, _ = run(trace=False, **inputs)
    return outputs
